# revision 24
# baseline (speedup 1.0000x reference)
"""Causal self-attention kernel for Trainium2 (8 NeuronCores, Bass/Tile).

Problem: B=4, S=2048, D=1024, H=16, HD=64, fp32.
Sharding: core c -> (batch b = c//2, head-group hg = c%2). Each core computes
attention for its batch over 8 heads (features hg*512..hg*512+511 of each of
the k/q/v projection chunks), plus the partial output projection
attn_out_slice @ W_out[rows of this head group].  Host sums the two partial
out-projections per batch and adds nothing else (b_out folded in on hg==0).

Device-side layout choices (no on-device transposes anywhere):
  - host provides x^T [D, S]; K^T/Q^T are produced feature-major [F, S] by
    using W as the matmul stationary operand; V is produced seq-major [S, F]
    by using x^T as the stationary operand.
  - attention uses the scores-transposed layout S^T[k, q]: QK^T pairs of
    heads run row-tiled (head A in PE rows 0-63, head B in rows 64-127),
    exp() on the scalar engine (no max subtraction: scores ~ N(0,1)),
    causal masking as a 0/1 multiply on band tiles only, AV col-tiled
    (head A -> PSUM partitions 0-63, head B -> 64-127), denominators via
    M=1 ones-matmuls, softmax division folded into the PSUM eviction.
"""

import math
import os
from contextlib import ExitStack

import numpy as np
from ml_dtypes import bfloat16

import concourse.bass as bass
import concourse.tile as tile
from concourse import bacc, mybir
from concourse.bass_utils import run_bass_kernel_spmd

F32 = mybir.dt.float32
BF16 = mybir.dt.bfloat16

# Matmul compute dtype: float32r (TF32-like, 1 cycle/row at N>=256) unless
# overridden for an accuracy fallback.
_MM_DT = {
    "f32r": mybir.dt.float32r,
    "f32": mybir.dt.float32,
}[os.environ.get("KERNEL_MM_DT", "f32r")]


def _r(ap):
    """Reinterpret an fp32 AP as the matmul compute dtype (same bytes)."""
    if _MM_DT == mybir.dt.float32:
        return ap
    return ap.bitcast(_MM_DT)


def build_nc(S=2048, D=1024, H_pc=8, HD=64, NQ=512, KT=128, reps=1):
    """Build the single-core Bass program (identical program on all cores).

    reps>1 wraps the whole kernel body in an on-device For_i loop; used only
    for slope-based HW timing (delta wall-time per rep through the axon
    tunnel), never for the graded path.
    """
    F = H_pc * HD          # per-core feature width of each of k/q/v (512)
    HP = F // 128          # head-pairs == 128-wide feature tiles (4)
    DKT = D // 128         # contraction tiles over d_model (8)
    NSEQ = S // NQ         # q blocks (4)
    NST = S // KT          # seq tiles for V (16)
    DM = D // 128          # output d_model tiles (8)
    NCH = S // 512         # 512-wide seq chunks for projections (4)
    BAND = NQ // KT        # k-tiles per q-block on the causal diagonal (4)

    nc = bacc.Bacc("TRN2", target_bir_lowering=False, debug=False, num_devices=8)

    x_t = nc.dram_tensor("x_t", [D, S], BF16, kind="ExternalInput").ap()
    w_k = nc.dram_tensor("w_k", [D, F], BF16, kind="ExternalInput").ap()
    w_q = nc.dram_tensor("w_q", [D, F], BF16, kind="ExternalInput").ap()
    w_v = nc.dram_tensor("w_v", [D, F], BF16, kind="ExternalInput").ap()
    b_k = nc.dram_tensor("b_k", [F, 1], F32, kind="ExternalInput").ap()
    b_q = nc.dram_tensor("b_q", [F, 1], F32, kind="ExternalInput").ap()
    b_v = nc.dram_tensor("b_v", [F], F32, kind="ExternalInput").ap()
    w_o = nc.dram_tensor("w_o", [F, D], BF16, kind="ExternalInput").ap()
    b_o = nc.dram_tensor("b_o", [D, 1], F32, kind="ExternalInput").ap()
    masks = nc.dram_tensor("masks", [128, 2, 128], BF16, kind="ExternalInput").ap()
    out_t = nc.dram_tensor("out_t", [D, S], F32, kind="ExternalOutput").ap()

    scale = 1.0 / math.sqrt(HD)

    with tile.TileContext(nc) as tc, ExitStack() as ctx:
        consts = ctx.enter_context(tc.tile_pool(name="consts", bufs=1))
        # per-partition bias columns for the feature-major K/Q projections
        bk_sb = consts.tile([128, HP], F32, tag="bk")
        bq_sb = consts.tile([128, HP], F32, tag="bq")
        nc.sync.dma_start(out=bk_sb, in_=b_k.rearrange("(m p) one -> p (m one)", p=128))
        nc.sync.dma_start(out=bq_sb, in_=b_q.rearrange("(m p) one -> p (m one)", p=128))
        # V bias broadcast along partitions (bias varies along the free dim)
        bv_sb = consts.tile([128, F], F32, tag="bv")
        bv_bcast = bass.AP(tensor=b_v.tensor, offset=b_v.offset, ap=[[0, 128], [1, F]])
        nc.sync.dma_start(out=bv_sb, in_=bv_bcast)
        bo_sb = consts.tile([128, DM], F32, tag="bo")
        nc.sync.dma_start(out=bo_sb, in_=b_o.rearrange("(m p) one -> p (m one)", p=128))
        # all-ones row block: lhsT slices for the 1/D broadcast matmuls
        onesw_f = consts.tile([128, HD], F32, tag="oneswf")
        nc.vector.memset(onesw_f, 1.0)
        onesw = consts.tile([128, HD], F32, tag="onesw")
        nc.vector.tensor_copy(_r(onesw), onesw_f)

        # persistent activations.  v stores 65 columns per head: 64 features
        # plus a constant-1 column, so the AV matmul's 65th output partition
        # accumulates the softmax denominator for free.
        HD1 = HD + 1
        FV = H_pc * HD1        # 520
        big = ctx.enter_context(tc.tile_pool(name="big", bufs=1))
        kT = [big.tile([128, S], F32, tag=f"kT{m}", name=f"kT{m}") for m in range(HP)]
        qT = [big.tile([128, S], F32, tag=f"qT{m}", name=f"qT{m}") for m in range(HP)]
        v = [big.tile([128, FV], BF16, tag=f"v{st}", name=f"v{st}") for st in range(NST)]
        aT = [big.tile([128, S], BF16, tag=f"aT{m}", name=f"aT{m}") for m in range(HP)]
        # strided memsets are not valid ISA: set whole tiles to 1.0 once; the
        # V-projection eviction overwrites the 64 feature columns per head and
        # leaves each head's 65th (denominator) column at 1.0.
        for st in range(NST):
            nc.vector.memset(v[st][:, :], 1.0)

        # ---- Phases A+B interleaved: V, then per head-pair {K,Q proj; attention} ----
        # All [128,512] PSUM accumulations (V-proj, K/Q-proj, scores) share one
        # 4-buffer pool so projection and attention pipelines coexist in the
        # 8 PSUM banks and the scheduler can overlap them across head-pairs.
        xp = ctx.enter_context(tc.tile_pool(name="xp", bufs=1))
        mk = ctx.enter_context(tc.tile_pool(name="mk", bufs=1))
        wsp = ctx.enter_context(tc.tile_pool(name="wsp", bufs=2 * DKT))
        ptp = ctx.enter_context(tc.tile_pool(name="pt_pool", bufs=6))
        r2p = ctx.enter_context(tc.tile_pool(name="r2_pool", bufs=2))
        sp = ctx.enter_context(tc.tile_pool(name="sp", bufs=2, space="PSUM"))
        op = ctx.enter_context(tc.tile_pool(name="op", bufs=2, space="PSUM"))
        wop = ctx.enter_context(tc.tile_pool(name="wop", bufs=1))
        osb = ctx.enter_context(tc.tile_pool(name="os", bufs=6))
        wvp = ctx.enter_context(tc.tile_pool(name="wvp", bufs=1))

        def body():
            maskt = mk.tile([128, 2, 128], BF16, tag="mask", name="maskt")
            nc.sync.dma_start(out=maskt, in_=masks)
            xt = [xp.tile([128, S], BF16, tag=f"x{k}", name=f"x{k}") for k in range(DKT)]

            # V first (its weights can be released before the K/Q W stream);
            # interleave wv/x loads so the first accumulation starts early
            wv = [wvp.tile([128, F], BF16, tag=f"wv{k}", name=f"wv{k}") for k in range(DKT)]
            for k in range(DKT):
                nc.sync.dma_start(out=wv[k], in_=w_v[k * 128:(k + 1) * 128, :])
            # split x loads by 512-col chunk so the first V matmuls start after
            # ~1/4 of the x traffic instead of all of it
            for c in range(NCH):
                cs = slice(c * 512, (c + 1) * 512)
                for k in range(DKT):
                    nc.sync.dma_start(out=xt[k][:, cs],
                                      in_=x_t[k * 128:(k + 1) * 128, cs])
            for st in range(NST):
                ps = sp.tile([128, 2 * NQ], F32, tag="s")
                for k in range(DKT):
                    nc.tensor.matmul(
                        ps[:, 0:F], xt[k][:, st * 128:(st + 1) * 128], wv[k],
                        start=(k == 0), stop=(k == DKT - 1),
                    )
                v3 = v[st].rearrange("p (h c) -> p h c", c=HD1)
                ps3 = ps[:, 0:F].rearrange("p (h c) -> p h c", c=HD)
                bv3 = bv_sb.rearrange("p (h c) -> p h c", c=HD)
                nc.vector.tensor_add(v3[:, :, 0:HD], ps3, bv3)

            cA = slice(0, 64)
            cB = slice(64, 128)

            # Output projection per 512-chunk through the shared score pool;
            # W_out for this core is 1 MB bf16: preload it fully.
            wo = [[wop.tile([128, 128], BF16, tag=f"wo{k}_{mo}", name=f"wo{k}_{mo}")
                   for mo in range(DM)] for k in range(HP)]
            for k in range(HP):
                for mo in range(DM):
                    nc.sync.dma_start(
                        out=wo[k][mo],
                        in_=w_o[k * 128:(k + 1) * 128, mo * 128:(mo + 1) * 128],
                    )

            def outproj_chunk(nch):
                for mo in range(DM):
                    ps = sp.tile([128, 2 * NQ], F32, tag="s")
                    for k in range(HP):
                        nc.tensor.matmul(
                            ps[:, 0:512], wo[k][mo],
                            aT[k][:, nch * 512:(nch + 1) * 512],
                            start=(k == 0), stop=(k == HP - 1),
                        )
                    ot = osb.tile([128, 512], F32, tag="ot")
                    nc.vector.tensor_scalar_add(ot, ps[:, 0:512], bo_sb[:, mo:mo + 1])
                    nc.sync.dma_start(
                        out=out_t[mo * 128:(mo + 1) * 128, nch * 512:(nch + 1) * 512],
                        in_=ot,
                    )

            pending = []  # deferred per-(hp, qi) eviction emitters

            for hp in range(HP):
                # K and Q projections for this head-pair's feature tile
                for (wdram, bias_sb, dstT) in ((w_k, bk_sb, kT), (w_q, bq_sb, qT)):
                    wt = [wsp.tile([128, 128], BF16, tag="w", name="wt") for _ in range(DKT)]
                    for k in range(DKT):
                        nc.sync.dma_start(
                            out=wt[k],
                            in_=wdram[k * 128:(k + 1) * 128, hp * 128:(hp + 1) * 128],
                        )
                    for nch in range(NCH):
                        ps = sp.tile([128, 2 * NQ], F32, tag="s")
                        for k in range(DKT):
                            nc.tensor.matmul(
                                ps[:, 0:512], wt[k], xt[k][:, nch * 512:(nch + 1) * 512],
                                start=(k == 0), stop=(k == DKT - 1),
                            )
                        nc.vector.tensor_scalar_add(
                            _r(dstT[hp][:, nch * 512:(nch + 1) * 512]), ps[:, 0:512],
                            bias_sb[:, hp:hp + 1],
                        )

                # attention for this head-pair
                for qi in range(NSEQ):
                    nkt = (qi + 1) * BAND
                    # o holds head A in bank 0 cols, head B in bank 1 cols;
                    # partition 64 of each accumulates the softmax denominator
                    # (the constant-1 column of v).
                    o = op.tile([128, 2 * NQ], F32, tag="o")
                    qs = slice(qi * NQ, (qi + 1) * NQ)
                    for kt in range(nkt):
                        ks = slice(kt * 128, (kt + 1) * 128)
                        j = kt - (nkt - BAND)
                        # valid q-subrange of this k-tile: q_local >= 128*j
                        lo = 128 * j if j > 0 else 0
                        w = NQ - lo
                        # fp32r needs moving dim >= 256 for full rate
                        qk_lo = lo if w >= 256 else 0
                        s2 = sp.tile([128, 2 * NQ], F32, tag="s")
                        qsub = slice(qi * NQ + qk_lo, (qi + 1) * NQ)
                        nc.tensor.matmul(
                            s2[:, qk_lo:NQ], _r(kT[hp][cA, ks]), _r(qT[hp][cA, qsub]),
                            start=True, stop=True, tile_position=(0, 0),
                        )
                        nc.tensor.matmul(
                            s2[:, NQ + qk_lo:2 * NQ], _r(kT[hp][cB, ks]),
                            _r(qT[hp][cB, qsub]),
                            start=True, stop=True, tile_position=(64, 0),
                        )
                        pt = ptp.tile([128, 2 * NQ], BF16, tag="p")
                        s2_3 = s2.rearrange("p (h q) -> p h q", h=2)
                        pt_3 = pt.rearrange("p (h q) -> p h q", h=2)
                        nc.scalar.activation(
                            pt_3[:, :, lo:NQ], s2_3[:, :, lo:NQ],
                            mybir.ActivationFunctionType.Exp, scale=scale,
                        )
                        if j >= 0:
                            # triangle mask on the first 128 valid columns
                            nc.vector.tensor_mul(
                                pt_3[:, :, lo:lo + 128], pt_3[:, :, lo:lo + 128],
                                maskt,
                            )
                        first, last = (kt == 0), (kt == nkt - 1)
                        nc.tensor.matmul(
                            o[0:65, lo:NQ],
                            v[kt][:, hp * 2 * HD1:hp * 2 * HD1 + HD1],
                            pt[:, lo:NQ],
                            start=first, stop=last, tile_position=(0, 0),
                            skip_group_check=True,
                        )
                        nc.tensor.matmul(
                            o[0:65, NQ + lo:2 * NQ],
                            v[kt][:, hp * 2 * HD1 + HD1:hp * 2 * HD1 + 2 * HD1],
                            pt[:, NQ + lo:2 * NQ],
                            start=first, stop=last, tile_position=(0, 0),
                            skip_group_check=True,
                        )
                    # 1/denominator (row 64 of each half) -> broadcast over the
                    # 64 feature rows via a K=1 ones matmul -> normalize+evict.
                    # Deferred one q-block (lag-1 software pipeline) so the PE
                    # never stalls waiting on the DVE reciprocal chain.
                    def evict(hp=hp, qi=qi, o=o, qs=qs):
                        r2v = r2p.tile([65, 2 * NQ], F32, tag="r2v")
                        # f32r out is byte-identical to f32; tag it so the
                        # f32r broadcast matmul accepts it as pre-rounded.
                        with nc.allow_low_precision(reason="f32r bitcast"):
                            nc.vector.reciprocal(
                                _r(r2v[64:65, :]), o[64:65, 0:2 * NQ])
                        re = sp.tile([128, 2 * NQ], F32, tag="s")
                        nc.tensor.matmul(
                            re[0:64, 0:NQ], _r(onesw[64:65, :]),
                            _r(r2v[64:65, 0:NQ]), start=True, stop=True,
                        )
                        nc.tensor.matmul(
                            re[0:64, NQ:2 * NQ], _r(onesw[64:65, :]),
                            _r(r2v[64:65, NQ:2 * NQ]), start=True, stop=True,
                        )
                        re_sb = r2p.tile([64, 2 * NQ], F32, tag="re_sb")
                        nc.vector.tensor_copy(re_sb, re[0:64, 0:2 * NQ])
                        nc.vector.tensor_mul(
                            aT[hp][cA, qs], o[0:64, 0:NQ], re_sb[:, 0:NQ])
                        stgB = r2p.tile([64, NQ], BF16, tag="stgB")
                        nc.vector.tensor_mul(
                            stgB, o[0:64, NQ:2 * NQ], re_sb[:, NQ:2 * NQ])
                        # partition shift 0-63 -> 64-127 (DVE can't cross lanes)
                        nc.sync.dma_start(out=aT[hp][cB, qs], in_=stgB)
                        if hp == HP - 1:
                            # all heads' chunk qi now evicted: output projection
                            outproj_chunk(qi)
                    pending.append(evict)
                    if len(pending) > 1:
                        pending.pop(0)()

            while pending:
                pending.pop(0)()

        if reps == 1:
            body()
        else:
            with tc.For_i(0, reps, 1):
                body()

    nc.compile()
    return nc


def make_masks(NQ=512, KT=128):
    # triangle mask for the 128-wide causal boundary, duplicated for 2 heads
    k = np.arange(128)[:, None]
    c = np.arange(128)[None, :]
    keep = (c >= k).astype(np.float32)
    return np.stack([keep, keep], axis=1)  # [128, 2, 128]


def make_in_maps(x, W_in, b_in, W_out, b_out, S, D, H_pc, HD):
    """Build the 8 per-core input maps. Core c -> (batch c//2, head-group c%2)."""
    F = H_pc * HD
    B = x.shape[0]
    n_hg = D // F  # 2
    masks = make_masks()
    in_maps = []
    for c in range(B * n_hg):
        b, hg = c // n_hg, c % n_hg
        cols = slice(hg * F, (hg + 1) * F)
        # W_in chunk order (torch.chunk in the reference): k, q, v
        wk = np.ascontiguousarray(W_in[:, 0 * D:1 * D][:, cols])
        wq = np.ascontiguousarray(W_in[:, 1 * D:2 * D][:, cols])
        wv = np.ascontiguousarray(W_in[:, 2 * D:3 * D][:, cols])
        bk = np.ascontiguousarray(b_in[0 * D:1 * D][cols]).reshape(F, 1)
        bq = np.ascontiguousarray(b_in[1 * D:2 * D][cols]).reshape(F, 1)
        bv = np.ascontiguousarray(b_in[2 * D:3 * D][cols])
        wo = np.ascontiguousarray(W_out[cols, :])
        bo = (b_out if hg == 0 else np.zeros_like(b_out)).reshape(D, 1)
        in_maps.append({
            "x_t": np.ascontiguousarray(x[b].T).astype(bfloat16),
            "w_k": wk.astype(bfloat16), "w_q": wq.astype(bfloat16),
            "w_v": wv.astype(bfloat16),
            "b_k": bk.astype(np.float32), "b_q": bq.astype(np.float32),
            "b_v": bv.astype(np.float32),
            "w_o": wo.astype(bfloat16), "b_o": bo.astype(np.float32),
            "masks": masks.astype(bfloat16),
        })
    return in_maps


_NC_CACHE = {}


def _get_nc(key, **kw):
    if key not in _NC_CACHE:
        _NC_CACHE[key] = build_nc(**kw)
    return _NC_CACHE[key]


def kernel(x, W_in, b_in, W_out, b_out):
    x = np.asarray(x, dtype=np.float32)
    W_in = np.asarray(W_in, dtype=np.float32)
    b_in = np.asarray(b_in, dtype=np.float32)
    W_out = np.asarray(W_out, dtype=np.float32)
    b_out = np.asarray(b_out, dtype=np.float32)

    B, S, D = x.shape          # 4, 2048, 1024
    HD = 64
    H_pc = (D // HD) // 2      # 8 heads per core

    nc = _get_nc((S, D, H_pc), S=S, D=D, H_pc=H_pc, HD=HD)
    in_maps = make_in_maps(x, W_in, b_in, W_out, b_out, S, D, H_pc, HD)
    res = run_bass_kernel_spmd(nc, in_maps, list(range(2 * B)))
    outs = res.results
    out = np.empty((B, S, D), dtype=np.float32)
    for b in range(B):
        out[b] = (outs[2 * b]["out_t"] + outs[2 * b + 1]["out_t"]).T
    return out


def _pjrt_runner(nc, n_cores):
    """Cached jitted 8-core runner with no donation, for steady-state timing."""
    import jax
    from jax.sharding import Mesh, PartitionSpec, NamedSharding
    from jax.experimental.shard_map import shard_map
    from concourse import bass2jax, mybir as mb
    bass2jax.install_neuronx_cc_hook()

    partition_name = nc.partition_id_tensor.name if nc.partition_id_tensor else None
    in_names, out_names, out_avals, zero_outs = [], [], [], []
    for alloc in nc.m.functions[0].allocations:
        if not isinstance(alloc, mb.MemoryLocationSet):
            continue
        name = alloc.memorylocations[0].name
        if alloc.kind == "ExternalInput":
            if name != partition_name:
                in_names.append(name)
        elif alloc.kind == "ExternalOutput":
            out_names.append(name)
            shape = tuple(alloc.tensor_shape)
            dtype = mb.dt.np(alloc.dtype)
            out_avals.append(jax.core.ShapedArray(shape, dtype))
            zero_outs.append(np.zeros(shape, dtype))
    n_params = len(in_names)
    all_names = in_names + out_names
    if partition_name is not None:
        all_names = all_names + [partition_name]

    def _body(*args):
        operands = list(args)
        if partition_name is not None:
            operands.append(bass2jax.partition_id_tensor())
        outs = bass2jax._bass_exec_p.bind(
            *operands,
            out_avals=tuple(out_avals),
            in_names=tuple(all_names),
            out_names=tuple(out_names),
            lowering_input_output_aliases=(),
            sim_require_finite=True,
            sim_require_nnan=True,
            nc=nc,
        )
        return tuple(outs)

    devices = jax.devices()[:n_cores]
    mesh = Mesh(np.asarray(devices), ("core",))
    spec = PartitionSpec("core")
    f = jax.jit(shard_map(
        _body, mesh=mesh,
        in_specs=(spec,) * (n_params + len(out_names)),
        out_specs=(spec,) * len(out_names),
        check_rep=False,
    ))
    sharding = NamedSharding(mesh, spec)
    return f, in_names, zero_outs, sharding, out_names


def time_kernel(x, W_in, b_in, W_out, b_out, iters=10):
    """Steady-state per-call wall time (ns) of the 8-core execution with
    device-resident inputs (no H2D in the timed region)."""
    import time as _time
    import jax
    x = np.asarray(x, dtype=np.float32)
    B, S, D = x.shape
    HD = 64
    H_pc = (D // HD) // 2
    nc = _get_nc((S, D, H_pc), S=S, D=D, H_pc=H_pc, HD=HD)
    in_maps = make_in_maps(np.asarray(x), np.asarray(W_in), np.asarray(b_in),
                           np.asarray(W_out), np.asarray(b_out), S, D, H_pc, HD)
    n_cores = len(in_maps)
    f, in_names, zero_outs, sharding, out_names = _pjrt_runner(nc, n_cores)
    args = []
    for name in in_names:
        g = np.concatenate([np.asarray(in_maps[c][name]) for c in range(n_cores)], axis=0)
        args.append(jax.device_put(g, sharding))
    for z in zero_outs:
        g = np.concatenate([z] * n_cores, axis=0)
        args.append(jax.device_put(g, sharding))
    out = f(*args)
    jax.block_until_ready(out)  # warmup + compile
    times = []
    for _ in range(iters):
        t0 = _time.perf_counter()
        out = f(*args)
        jax.block_until_ready(out)
        times.append(_time.perf_counter() - t0)
    return min(times) * 1e9

